# revision 9
# baseline (speedup 1.0000x reference)
"""CRaWl GNN forward on 8 Trainium2 NeuronCores (Bass/Tile).

Sharding: walks 6250/core for the gather+conv path; nodes 6250/core (padded
slices of 6272) for the node/VN stage; fp16 node table replicated via
AllGather; scatter_mean via host-sorted one-hot matmuls + ReduceScatter;
BN batch stats via small AllReduces.
"""

import os
import sys

sys.path.insert(0, "/opt/trn_rl_repo")

import numpy as np

import concourse.bacc as bacc
import concourse.bass as bass
import concourse.mybir as mybir
import concourse.tile as tile
from concourse.bass import IndirectOffsetOnAxis
from concourse.bass_utils import run_bass_kernel_spmd

F16 = mybir.dt.float16
F32 = mybir.dt.float32
I32 = mybir.dt.int32
AF = mybir.ActivationFunctionType
OP = mybir.AluOpType

NC = 8
N_NODES, N_WALKS, L, KK, POOL = 50000, 50000, 17, 9, 4
EDGE_F, WALK_F, HID, CONV = 16, 16, 128, 128
N_GRAPHS, EPS = 256, 1e-5
NREAL = 6250
NSLICE = 6272
NPADN = NC * NSLICE
NJ = L - KK + 1

WSH = int(os.environ.get("KWSH", "6250"))
CW = int(os.environ.get("KCW", "256"))
NST = NC * WSH * NJ


def _chunks():
    n_full, rem = divmod(WSH, CW)
    return [CW] * n_full + ([rem] if rem else [])


def g2row(node):
    c = np.minimum(node // NREAL, NC - 1)
    return c * NSLICE + (node - c * NREAL)


def _scatter_plan(nodes_of_item):
    order = np.argsort(nodes_of_item, kind="stable")
    sn = nodes_of_item[order]
    n = len(order)
    slots = np.zeros(n, np.int64)
    slot = 0
    i = 0
    while i < n:
        j = i
        v = sn[i]
        while j < n and sn[j] == v:
            j += 1
        cnt = j - i
        if (slot % 128) + cnt > 128:
            slot = ((slot // 128) + 1) * 128
        slots[i:j] = slot + np.arange(cnt)
        slot += cnt
        i = j
    nslots = ((slot + 127) // 128) * 128
    dest = np.zeros(n, np.int64)
    dest[order] = slots
    slot_node = np.full(nslots, -1, np.int64)
    slot_node[slots] = sn
    return dest, nslots, slot_node


def _build(nslots_max, repeat=1, rs_f16=True):
    NBLK = nslots_max // 128
    chunks = _chunks()
    NBLKW = sum((NJ * w + 127) // 128 for w in chunks)
    NPOSP = sum(((17 * w + 127) // 128) * 128 for w in chunks)
    NCALLS = NPOSP // 128
    NIT = WSH * NJ
    NCH_N = (NSLICE + 511) // 512
    pn_dt = F16 if rs_f16 else F32

    nc = bacc.Bacc("TRN2")
    dt = nc.dram_tensor
    xin0 = dt("xin0", [64, NPOSP], F16, kind="ExternalInput")
    gidx = dt("gidx", [128, NCALLS], I32, kind="ExternalInput")
    widx = dt("widx", [128, NBLKW], I32, kind="ExternalInput")
    ridx = dt("ridx", [128, NBLK], I32, kind="ExternalInput")
    ssrt = dt("ssrt", [NBLK, 128, 128], F16, kind="ExternalInput")
    xts = dt("xts", [32, NSLICE], F16, kind="ExternalInput")
    bmat = dt("bmat", [NSLICE, N_GRAPHS], F16, kind="ExternalInput")
    btm = dt("btm", [N_GRAPHS, NSLICE], F16, kind="ExternalInput")
    idn = dt("idn", [128, 128], F16, kind="ExternalInput")
    w16 = dt("w16", [128, 3328], F16, kind="ExternalInput")
    w32 = dt("w32", [128, 64], F32, kind="ExternalInput")
    out_h = dt("out_h", [128, NSLICE], F32, kind="ExternalOutput")

    table = dt("table", [NPADN, 128], F16, kind="Internal", addr_space="Shared")
    tslice = dt("tslice", [NSLICE, 128], F16, kind="Internal")
    yf_rows = ((nslots_max + 128 + 2047) // 2048) * 2048
    yf = dt("yf", [yf_rows, 128], F16, kind="Internal")
    udram = dt("udram", [128, NIT], F16, kind="Internal")
    pn = dt("pn", [NPADN, 128], pn_dt, kind="Internal")
    pns = dt("pns", [NSLICE, 128], pn_dt, kind="Internal")
    st_in = dt("st_in", [128, 8], F32, kind="Internal")
    st_out = dt("st_out", [128, 8], F32, kind="Internal", addr_space="Shared")
    g_in = dt("g_in", [128, N_GRAPHS], F32, kind="Internal")
    g_out = dt("g_out", [128, N_GRAPHS], F32, kind="Internal", addr_space="Shared")
    RG = [list(range(NC))]

    # --- weight pack layouts (all <=128 partitions) ---
    W16, W32 = {}, {}
    col = [0]
    def w16slot(name, parts, cols):
        W16[name] = (col[0], parts, cols)
        col[0] += cols
    col32 = [0]
    def w32slot(name, cols):
        W32[name] = (col32[0], cols)
        col32[0] += cols
    for li in range(3):
        w16slot(f"w1h{li}", 64 if li == 0 else 128, 128)
        if li:
            w16slot(f"w1e{li}", 32, 128)
        w16slot(f"w3t{li}", 128, 128)
        w16slot(f"o1{li}", 128, 256)
        w16slot(f"o2a{li}", 128, 128)
        w16slot(f"o2b{li}", 128, 128)
    w16slot("resc", 32, 128)
    for vi in range(2):
        w16slot(f"l1{vi}", 128, 128)
        w16slot(f"l2{vi}", 128, 128)
    assert col[0] <= 3328, col
    for li in range(3):
        w32slot(f"bn1g{li}", 1); w32slot(f"bn1b{li}", 1)
        w32slot(f"obng{li}", 2); w32slot(f"obnb{li}", 2)
        w32slot(f"dw{li}", 9)
    for vi in range(2):
        w32slot(f"vbg{vi}", 1); w32slot(f"vbb{vi}", 1)
    w32slot("fing", 1); w32slot("finb", 1)
    assert col32[0] <= 64

    with tile.TileContext(nc) as tc:
        with tc.tile_pool(name="persist", bufs=1) as pp:
            wt = pp.tile([128, 3328], F16)
            nc.sync.dma_start(wt[:], w16[:])
            wv = pp.tile([128, 64], F32)
            nc.sync.dma_start(wv[:], w32[:])
            idt = pp.tile([128, 128], F16)
            nc.sync.dma_start(idt[:], idn[:])
            gix = pp.tile([128, NCALLS], I32)
            nc.sync.dma_start(gix[:], gidx[:])
            wix = pp.tile([128, NBLKW], I32)
            nc.sync.dma_start(wix[:], widx[:])
            rix = pp.tile([128, NBLK], I32)
            nc.sync.dma_start(rix[:], ridx[:])
            xtt = pp.tile([32, NSLICE], F16)
            nc.sync.dma_start(xtt[:], xts[:])
            zt16 = pp.tile([128, 16, 128], pn_dt)
            nc.vector.memset(zt16[:], 0.0)
            epst = pp.tile([128, 1], F32)
            nc.vector.memset(epst[:], float(EPS))

            hT = pp.tile([128, NSLICE], F32)
            vnh = pp.tile([128, N_GRAPHS], F32)
            vnhT = pp.tile([128, 2, 128], F16)
            pnT = pp.tile([128, NSLICE], F16)
            h16 = pp.tile([128, NSLICE], F16)
            tv = pp.tile([128, NSLICE], F16)
            sc1 = pp.tile([128, 1], F32); bi1 = pp.tile([128, 1], F32)
            sc2 = pp.tile([128, 2], F32); bi2 = pp.tile([128, 2], F32)

            def ws(name):
                c0, p, w = W16[name]
                return wt[:p, c0:c0 + w]

            def wv1(name):
                c0, w = W32[name]
                return wv[:, c0:c0 + w]

            def bnstats(sp, nsum, nsq, count, gv, bv, scd, bid, half):
                mean = sp.tile([128, 1], F32, tag="bs1")
                nc.scalar.mul(mean[:], nsum, 1.0 / count)
                m2 = sp.tile([128, 1], F32, tag="bs2")
                nc.vector.tensor_tensor(out=m2[:], in0=mean[:], in1=mean[:], op=OP.mult)
                var = sp.tile([128, 1], F32, tag="bs3")
                nc.scalar.mul(var[:], nsq, 1.0 / count)
                nc.vector.tensor_tensor(out=var[:], in0=var[:], in1=m2[:], op=OP.subtract)
                sd = sp.tile([128, 1], F32, tag="bs4")
                nc.scalar.activation(sd[:], var[:], AF.Sqrt, bias=epst[:])
                rs_ = sp.tile([128, 1], F32, tag="bs5")
                nc.vector.reciprocal(rs_[:], sd[:])
                nc.vector.tensor_tensor(out=scd[:, half:half + 1], in0=gv, in1=rs_[:], op=OP.mult)
                mb = sp.tile([128, 1], F32, tag="bs6")
                nc.vector.tensor_tensor(out=mb[:], in0=mean[:], in1=scd[:, half:half + 1], op=OP.mult)
                nc.vector.tensor_tensor(out=bid[:, half:half + 1], in0=bv, in1=mb[:], op=OP.subtract)

            for rep in range(repeat):
                zf16 = pp.tile([128, 16, 128], F16, tag="zf16")
                nc.vector.memset(zf16[:], 0.0)
                for zz in range(yf_rows // 2048):
                    nc.sync.dma_start(
                        yf.ap().rearrange("(a p) f -> p a f", p=128)[:, 16 * zz:16 * (zz + 1), :],
                        zf16[:])
                for li in range(3):
                    # ============ pass A ============
                    with tc.tile_pool(name=f"pab{li}{rep}", bufs=2) as sp, \
                         tc.tile_pool(name=f"pas{li}{rep}", bufs=4) as sps, \
                         tc.tile_pool(name=f"pap{li}{rep}", bufs=2, space="PSUM") as qp:
                        stc = sp.tile([128, 32], F32, tag="stc")
                        stc2 = sp.tile([128, 32], F32, tag="stc2")
                        call0 = 0
                        ubase = 0
                        for ci, w in enumerate(chunks):
                            npos = 17 * w
                            nposp = ((npos + 127) // 128) * 128
                            ncall = nposp // 128
                            if li == 0:
                                xh = sp.tile([64, 17 * CW], F16, tag="xh0")
                                nc.sync.dma_start(xh[:, :nposp],
                                                  xin0[:, 128 * call0:128 * call0 + nposp])
                            else:
                                xh = sp.tile([128, 17 * CW], F16, tag="xh")
                                for k in range(ncall):
                                    gt = sps.tile([128, 128], F16, tag="gat")
                                    nc.gpsimd.indirect_dma_start(
                                        gt[:], None, table[:],
                                        IndirectOffsetOnAxis(ap=gix[:, call0 + k:call0 + k + 1], axis=0))
                                    tps = qp.tile([128, 128], F16, tag="tp")
                                    nc.tensor.transpose(tps[:], gt[:], idt[:])
                                    nc.scalar.copy(xh[:, 128 * k:128 * (k + 1)], tps[:])
                                xe = sp.tile([32, 17 * CW], F16, tag="xe")
                                nc.sync.dma_start(xe[:, :nposp],
                                                  xin0[32:64, 128 * call0:128 * call0 + nposp])
                            va = sp.tile([128, 17 * CW], F16, tag="va")
                            for lp in range(17):
                                ps = qp.tile([128, CW], F32, tag="c1")
                                if li == 0:
                                    nc.tensor.matmul(ps[:, :w], ws("w1h0"), xh[:, lp * w:(lp + 1) * w],
                                                     start=True, stop=True)
                                else:
                                    nc.tensor.matmul(ps[:, :w], ws(f"w1h{li}"), xh[:, lp * w:(lp + 1) * w],
                                                     start=True, stop=False)
                                    nc.tensor.matmul(ps[:, :w], ws(f"w1e{li}"), xe[:, lp * w:(lp + 1) * w],
                                                     start=False, stop=True)
                                nc.scalar.copy(va[:, lp * w:(lp + 1) * w], ps[:, :w])
                            nit = NJ * w
                            pa = sp.tile([128, NJ * CW], F16, tag="dwa")
                            pb = sp.tile([128, NJ * CW], F16, tag="dwb")
                            cur, nxt = pa, pb
                            nc.vector.tensor_scalar_mul(cur[:, :nit], va[:, 0:nit], wv1(f"dw{li}")[:, 0:1])
                            uo = sp.tile([128, NJ * CW], F16, tag="uo")
                            for k in range(1, KK):
                                dst = uo[:, :nit] if k == KK - 1 else nxt[:, :nit]
                                nc.vector.scalar_tensor_tensor(
                                    dst, va[:, k * w:k * w + nit], wv1(f"dw{li}")[:, k:k + 1],
                                    cur[:, :nit], OP.mult, OP.add)
                                cur, nxt = nxt, cur
                            nc.sync.dma_start(udram[:, ubase:ubase + nit], uo[:, :nit])
                            scr = sp.tile([128, NJ * CW], F16, tag="scr")
                            nc.scalar.activation(scr[:, :nit], uo[:, :nit], AF.Copy,
                                                 accum_out=stc[:, ci:ci + 1])
                            nc.scalar.activation(scr[:, :nit], uo[:, :nit], AF.Square,
                                                 accum_out=stc2[:, ci:ci + 1])
                            call0 += ncall
                            ubase += nit
                        s1 = sp.tile([128, 8], F32, tag="s18")
                        nc.vector.memset(s1[:], 0.0)
                        nc.vector.tensor_reduce(s1[:, 0:1], stc[:, :len(chunks)],
                                                axis=mybir.AxisListType.X, op=OP.add)
                        nc.vector.tensor_reduce(s1[:, 1:2], stc2[:, :len(chunks)],
                                                axis=mybir.AxisListType.X, op=OP.add)
                        nc.sync.dma_start(st_in[:], s1[:])
                        nc.gpsimd.collective_compute("AllReduce", OP.add, RG, [st_in[:]], [st_out[:]])
                        sr = sp.tile([128, 8], F32, tag="sr8")
                        nc.sync.dma_start(sr[:], st_out[:])
                        bnstats(sp, sr[:, 0:1], sr[:, 1:2], NST,
                                wv1(f"bn1g{li}"), wv1(f"bn1b{li}"), sc1, bi1, 0)
                        pnv = pn.ap().rearrange("(a p) f -> p a f", p=128)
                        for z0 in range(0, NPADN // 128, 16):
                            zn = min(16, NPADN // 128 - z0)
                            nc.sync.dma_start(pnv[:, z0:z0 + zn, :], zt16[:, :zn, :])
                    # ============ pass B ============
                    with tc.tile_pool(name=f"pbb{li}{rep}", bufs=2) as sp, \
                         tc.tile_pool(name=f"pbs{li}{rep}", bufs=4) as sps, \
                         tc.tile_pool(name=f"pbp{li}{rep}", bufs=2, space="PSUM") as qp:
                        ubase = 0
                        blk = 0
                        for ci, w in enumerate(chunks):
                            nit = NJ * w
                            nbl = (nit + 127) // 128
                            ur = sp.tile([128, NJ * CW], F16, tag="ur")
                            nc.sync.dma_start(ur[:, :nit], udram[:, ubase:ubase + nit])
                            y1 = sp.tile([128, NJ * CW], F16, tag="y1")
                            nc.scalar.activation(y1[:, :nit], ur[:, :nit], AF.Relu,
                                                 bias=bi1[:, 0:1], scale=sc1[:, 0:1])
                            if nit < nbl * 128:
                                nc.vector.memset(y1[:, nit:nbl * 128], 0.0)
                            for b0 in range(0, nbl, 4):
                                bn_ = min(4, nbl - b0)
                                ps = qp.tile([128, 512], F32, tag="c3")
                                stg = sps.tile([128, 4, 128], F16, tag="c3s")
                                for b in range(b0, b0 + bn_):
                                    nc.tensor.matmul(ps[:, 128 * (b - b0):128 * (b - b0 + 1)],
                                                     y1[:, 128 * b:128 * (b + 1)],
                                                     ws(f"w3t{li}"), start=True, stop=True)
                                nc.scalar.activation(stg[:, :bn_, :], ps[:, :bn_ * 128], AF.Relu)
                                for b in range(b0, b0 + bn_):
                                    nc.gpsimd.indirect_dma_start(
                                        yf[:], IndirectOffsetOnAxis(ap=wix[:, blk + b:blk + b + 1], axis=0),
                                        stg[:, b - b0, :], None)
                            ubase += nit
                            blk += nbl
                        assert blk == NBLKW, (blk, NBLKW)
                    # ============ pass C ============
                    with tc.tile_pool(name=f"pcb{li}{rep}", bufs=4) as sps, \
                         tc.tile_pool(name=f"pcp{li}{rep}", bufs=2, space="PSUM") as qp:
                        for b0 in range(0, NBLK, 4):
                            bn_ = min(4, NBLK - b0)
                            xs = sps.tile([128, 4, 128], F16, tag="xs")
                            nc.sync.dma_start(
                                xs[:, :bn_, :],
                                yf.ap().rearrange("(a p) f -> p a f", p=128)[:, b0:b0 + bn_, :])
                            sb_ = sps.tile([128, 4, 128], F16, tag="sbk")
                            nc.sync.dma_start(sb_[:, :bn_, :],
                                              ssrt.ap().rearrange("a p f -> p a f")[:, b0:b0 + bn_, :])
                            ps = qp.tile([128, 512], F32, tag="spp")
                            for b in range(bn_):
                                nc.tensor.matmul(ps[:, 128 * b:128 * (b + 1)], sb_[:, b, :], xs[:, b, :],
                                                 start=True, stop=True)
                            og = sps.tile([128, 4, 128], pn_dt, tag="og")
                            nc.scalar.activation(og[:, :bn_, :], ps[:, :bn_ * 128], AF.Copy)
                            for b in range(bn_):
                                nc.gpsimd.indirect_dma_start(
                                    pn[:], IndirectOffsetOnAxis(ap=rix[:, b0 + b:b0 + b + 1], axis=0),
                                    og[:, b, :], None)
                        nc.gpsimd.collective_compute("ReduceScatter", OP.add, RG, [pn[:]], [pns[:]])
                    # ============ node stage ============
                    with tc.tile_pool(name=f"ndb{li}{rep}", bufs=2) as sp, \
                         tc.tile_pool(name=f"nds{li}{rep}", bufs=4) as sps, \
                         tc.tile_pool(name=f"ndp{li}{rep}", bufs=1, space="PSUM") as qp:
                        for b0 in range(0, NSLICE // 128, 4):
                            bn_ = min(4, NSLICE // 128 - b0)
                            pr = sps.tile([128, 4, 128], pn_dt, tag="pr")
                            nc.sync.dma_start(pr[:, :bn_, :],
                                              pns.ap().rearrange("(a p) f -> p a f", p=128)[:, b0:b0 + bn_, :])
                            ps = qp.tile([128, 512], pn_dt, tag="np16")
                            for b in range(bn_):
                                nc.tensor.transpose(ps[:, 128 * b:128 * (b + 1)], pr[:, b, :], idt[:])
                            nc.scalar.copy(pnT[:, 128 * b0:128 * b0 + bn_ * 128], ps[:, :bn_ * 128])
                        # pass 1: z stats
                        stz = sp.tile([128, 8], F32, tag="stz")
                        zsc = sp.tile([128, 4096], F16, tag="zsc")
                        for half in range(2):
                            zrow = sp.tile([128, NSLICE], F32, tag="zrow")
                            for cb in range(NCH_N):
                                c0, c1 = 512 * cb, min(NSLICE, 512 * (cb + 1))
                                ps = qp.tile([128, 512], F32, tag="np")
                                nc.tensor.matmul(ps[:, :c1 - c0], ws(f"o1{li}")[:, 128 * half:128 * (half + 1)],
                                                 pnT[:, c0:c1], start=True, stop=True)
                                nc.vector.tensor_copy(zrow[:, c0:c1], ps[:, :c1 - c0])
                            nc.scalar.activation(zsc[:, :4096], zrow[:, :4096], AF.Copy,
                                                 accum_out=stz[:, 4 * half:4 * half + 1])
                            nc.scalar.activation(zsc[:, :NREAL - 4096], zrow[:, 4096:NREAL], AF.Copy,
                                                 accum_out=stz[:, 4 * half + 1:4 * half + 2])
                            nc.scalar.activation(zsc[:, :4096], zrow[:, :4096], AF.Square,
                                                 accum_out=stz[:, 4 * half + 2:4 * half + 3])
                            nc.scalar.activation(zsc[:, :NREAL - 4096], zrow[:, 4096:NREAL], AF.Square,
                                                 accum_out=stz[:, 4 * half + 3:4 * half + 4])
                        st2 = sp.tile([128, 8], F32, tag="st2")
                        nc.vector.memset(st2[:], 0.0)
                        nc.vector.tensor_tensor(out=st2[:, 0:1], in0=stz[:, 0:1], in1=stz[:, 1:2], op=OP.add)
                        nc.vector.tensor_tensor(out=st2[:, 1:2], in0=stz[:, 2:3], in1=stz[:, 3:4], op=OP.add)
                        nc.vector.tensor_tensor(out=st2[:, 2:3], in0=stz[:, 4:5], in1=stz[:, 5:6], op=OP.add)
                        nc.vector.tensor_tensor(out=st2[:, 3:4], in0=stz[:, 6:7], in1=stz[:, 7:8], op=OP.add)
                        nc.sync.dma_start(st_in[:], st2[:])
                        nc.gpsimd.collective_compute("AllReduce", OP.add, RG, [st_in[:]], [st_out[:]])
                        sr2 = sp.tile([128, 8], F32, tag="sr2")
                        nc.sync.dma_start(sr2[:], st_out[:])
                        for half in range(2):
                            bnstats(sp, sr2[:, 2 * half:2 * half + 1], sr2[:, 2 * half + 1:2 * half + 2],
                                    N_NODES, wv1(f"obng{li}")[:, half:half + 1],
                                    wv1(f"obnb{li}")[:, half:half + 1], sc2, bi2, half)
                        # pass 2: recompute z, relu-affine, h update
                        for cb in range(NCH_N):
                            c0, c1 = 512 * cb, min(NSLICE, 512 * (cb + 1))
                            zps = qp.tile([128, 2, 512], F32, tag="z2")
                            z16 = sps.tile([128, 2, 512], F16, tag="z16")
                            for half in range(2):
                                nc.tensor.matmul(zps[:, half, :c1 - c0],
                                                 ws(f"o1{li}")[:, 128 * half:128 * (half + 1)],
                                                 pnT[:, c0:c1], start=True, stop=True)
                                nc.scalar.activation(z16[:, half, :c1 - c0], zps[:, half, :c1 - c0],
                                                     AF.Relu, bias=bi2[:, half:half + 1],
                                                     scale=sc2[:, half:half + 1])
                            ps = qp.tile([128, 512], F32, tag="np")
                            nc.tensor.matmul(ps[:, :c1 - c0], ws(f"o2a{li}"), z16[:, 0, :c1 - c0],
                                             start=True, stop=False)
                            nc.tensor.matmul(ps[:, :c1 - c0], ws(f"o2b{li}"), z16[:, 1, :c1 - c0],
                                             start=False, stop=(li > 0))
                            if li == 0:
                                nc.tensor.matmul(ps[:, :c1 - c0], ws("resc"), xtt[:, c0:c1],
                                                 start=False, stop=True)
                                nc.scalar.copy(hT[:, c0:c1], ps[:, :c1 - c0])
                            else:
                                nc.vector.tensor_tensor(out=hT[:, c0:c1], in0=hT[:, c0:c1],
                                                        in1=ps[:, :c1 - c0], op=OP.add)
                        nc.vector.memset(hT[:, NREAL:], 0.0)
                        if li < 2:
                            nc.scalar.copy(h16[:], hT[:])
                            for b0 in range(0, NSLICE // 128, 4):
                                bn_ = min(4, NSLICE // 128 - b0)
                                ps = qp.tile([128, 512], F16, tag="np16")
                                for b in range(b0, b0 + bn_):
                                    nc.tensor.transpose(ps[:, 128 * (b - b0):128 * (b - b0 + 1)],
                                                        h16[:, 128 * b:128 * (b + 1)], idt[:])
                                nc.scalar.copy(tv[:, 128 * b0:128 * b0 + bn_ * 128], ps[:, :bn_ * 128])
                            gps = qp.tile([128, N_GRAPHS], F32, tag="gps")
                            for b in range(NSLICE // 128):
                                bblk = sps.tile([128, N_GRAPHS], F16, tag="bblk")
                                nc.sync.dma_start(bblk[:], bmat[128 * b:128 * (b + 1), :])
                                nc.tensor.matmul(gps[:], tv[:, 128 * b:128 * (b + 1)], bblk[:],
                                                 start=(b == 0), stop=(b == NSLICE // 128 - 1))
                            gsb = sp.tile([128, N_GRAPHS], F32, tag="gsb")
                            if li == 0:
                                nc.vector.tensor_copy(gsb[:], gps[:])
                            else:
                                nc.vector.tensor_tensor(out=gsb[:], in0=gps[:], in1=vnh[:], op=OP.add)
                            nc.sync.dma_start(g_in[:], gsb[:])
                            nc.gpsimd.collective_compute("AllReduce", OP.add, RG, [g_in[:]], [g_out[:]])
                            gT = sp.tile([128, N_GRAPHS], F32, tag="gT")
                            nc.sync.dma_start(gT[:], g_out[:])
                            g16 = sp.tile([128, N_GRAPHS], F16, tag="g16")
                            nc.vector.tensor_copy(g16[:], gT[:])
                            zps2 = qp.tile([128, N_GRAPHS], F32, tag="npz")
                            nc.tensor.matmul(zps2[:], ws(f"l1{li}"), g16[:], start=True, stop=True)
                            zvt = sp.tile([128, N_GRAPHS], F32, tag="zvt")
                            nc.vector.tensor_copy(zvt[:], zps2[:])
                            svn = sp.tile([128, 4], F32, tag="svn")
                            scrv = sp.tile([128, N_GRAPHS], F16, tag="scrv")
                            nc.scalar.activation(scrv[:], zvt[:], AF.Copy, accum_out=svn[:, 0:1])
                            nc.scalar.activation(scrv[:], zvt[:], AF.Square, accum_out=svn[:, 1:2])
                            scv = sp.tile([128, 1], F32, tag="scv")
                            biv = sp.tile([128, 1], F32, tag="biv")
                            bnstats(sp, svn[:, 0:1], svn[:, 1:2], N_GRAPHS,
                                    wv1(f"vbg{li}"), wv1(f"vbb{li}"), scv, biv, 0)
                            vt16 = sp.tile([128, N_GRAPHS], F16, tag="vt16")
                            nc.scalar.activation(vt16[:], zvt[:], AF.Relu, bias=biv[:, 0:1], scale=scv[:, 0:1])
                            vps = qp.tile([128, N_GRAPHS], F32, tag="npv")
                            nc.tensor.matmul(vps[:], ws(f"l2{li}"), vt16[:], start=True, stop=True)
                            nc.vector.tensor_copy(vnh[:], vps[:])
                            vh16 = sp.tile([128, N_GRAPHS], F16, tag="vh16")
                            nc.scalar.copy(vh16[:], vps[:])
                            for half in range(2):
                                pst = qp.tile([128, 128], F16, tag="np16b")
                                nc.tensor.transpose(pst[:], vh16[:, 128 * half:128 * (half + 1)], idt[:])
                                nc.scalar.copy(vnhT[:, half, :], pst[:])
                            for cb in range(NCH_N):
                                c0, c1 = 512 * cb, min(NSLICE, 512 * (cb + 1))
                                bt1 = sps.tile([128, 512], F16, tag="bt1")
                                bt2 = sps.tile([128, 512], F16, tag="bt2")
                                nc.sync.dma_start(bt1[:, :c1 - c0], btm[0:128, c0:c1])
                                nc.sync.dma_start(bt2[:, :c1 - c0], btm[128:256, c0:c1])
                                ps = qp.tile([128, 512], F32, tag="np")
                                nc.tensor.matmul(ps[:, :c1 - c0], vnhT[:, 0, :], bt1[:, :c1 - c0],
                                                 start=True, stop=False)
                                nc.tensor.matmul(ps[:, :c1 - c0], vnhT[:, 1, :], bt2[:, :c1 - c0],
                                                 start=False, stop=True)
                                nc.vector.tensor_tensor(out=hT[:, c0:c1], in0=hT[:, c0:c1],
                                                        in1=ps[:, :c1 - c0], op=OP.add)
                            nc.vector.memset(hT[:, NREAL:], 0.0)
                            nc.scalar.copy(h16[:], hT[:])
                            for b0 in range(0, NSLICE // 128, 4):
                                bn_ = min(4, NSLICE // 128 - b0)
                                ps = qp.tile([128, 512], F16, tag="np16")
                                stg = sps.tile([128, 4, 128], F16, tag="tstg")
                                for b in range(b0, b0 + bn_):
                                    nc.tensor.transpose(ps[:, 128 * (b - b0):128 * (b - b0 + 1)],
                                                        h16[:, 128 * b:128 * (b + 1)], idt[:])
                                nc.scalar.copy(stg[:, :bn_, :], ps[:, :bn_ * 128])
                                nc.sync.dma_start(
                                    tslice.ap().rearrange("(a p) f -> p a f", p=128)[:, b0:b0 + bn_, :],
                                    stg[:, :bn_, :])
                            nc.gpsimd.collective_compute("AllGather", OP.bypass, RG, [tslice[:]], [table[:]])
                # ============ final ============
                with tc.tile_pool(name=f"fin{rep}", bufs=2) as sp:
                    stf = sp.tile([128, 8], F32, tag="stf")
                    scrf = sp.tile([128, 4096], F16, tag="scrf")
                    nc.scalar.activation(scrf[:, :4096], hT[:, :4096], AF.Copy, accum_out=stf[:, 0:1])
                    nc.scalar.activation(scrf[:, :NREAL - 4096], hT[:, 4096:NREAL], AF.Copy,
                                         accum_out=stf[:, 1:2])
                    nc.scalar.activation(scrf[:, :4096], hT[:, :4096], AF.Square, accum_out=stf[:, 2:3])
                    nc.scalar.activation(scrf[:, :NREAL - 4096], hT[:, 4096:NREAL], AF.Square,
                                         accum_out=stf[:, 3:4])
                    st3 = sp.tile([128, 8], F32, tag="st3")
                    nc.vector.memset(st3[:], 0.0)
                    nc.vector.tensor_tensor(out=st3[:, 0:1], in0=stf[:, 0:1], in1=stf[:, 1:2], op=OP.add)
                    nc.vector.tensor_tensor(out=st3[:, 1:2], in0=stf[:, 2:3], in1=stf[:, 3:4], op=OP.add)
                    nc.sync.dma_start(st_in[:], st3[:])
                    nc.gpsimd.collective_compute("AllReduce", OP.add, RG, [st_in[:]], [st_out[:]])
                    srf = sp.tile([128, 8], F32, tag="srf")
                    nc.sync.dma_start(srf[:], st_out[:])
                    scf = sp.tile([128, 1], F32, tag="scf")
                    bif = sp.tile([128, 1], F32, tag="bif")
                    bnstats(sp, srf[:, 0:1], srf[:, 1:2], N_NODES, wv1("fing"), wv1("finb"), scf, bif, 0)
                    ot = sp.tile([128, NSLICE], F32, tag="ot")
                    nc.scalar.activation(ot[:], hT[:], AF.Relu, bias=bif[:, 0:1], scale=scf[:, 0:1])
                    nc.sync.dma_start(out_h[:], ot[:])
    nc.compile()
    return nc


def make_inmaps(inputs):
    """Host preprocessing: returns (per_core_inputs, nslots_max)."""
    x = np.asarray(inputs["x"], np.float32)
    edge_attr = np.asarray(inputs["edge_attr"], np.float32)
    walk_x = np.asarray(inputs["walk_x"], np.float32)
    wn = np.asarray(inputs["walk_nodes"], np.int64)
    we = np.asarray(inputs["walk_edges"], np.int64)
    batch = np.asarray(inputs["batch"], np.int64)
    params = inputs["params"]
    P32 = lambda t: np.asarray(t, np.float32)
    chunks = _chunks()
    NIT = WSH * NJ

    idx_center = wn[:, POOL:POOL + NJ]
    cnt = np.zeros(N_NODES, np.float64)
    np.add.at(cnt, idx_center[:NC * WSH].reshape(-1), 1.0)
    recip = (1.0 / np.maximum(cnt, 1.0)).astype(np.float32)

    ew_full = np.zeros((N_WALKS, L, 32), np.float32)
    ew_full[:, 1:, :EDGE_F] = edge_attr[we]
    ew_full[:, :, EDGE_F:] = walk_x.transpose(0, 2, 1)

    NPOSP = sum(((17 * w + 127) // 128) * 128 for w in chunks)
    NCALLS = NPOSP // 128
    plans = []
    for c in range(NC):
        w0 = c * WSH
        pos_nodes, item_nodes = [], []
        base = 0
        for w in chunks:
            wr = np.arange(w0 + base, w0 + base + w)
            pos_nodes.append(wn[wr].T.reshape(-1))
            pad = ((17 * w + 127) // 128) * 128 - 17 * w
            if pad:
                pos_nodes.append(np.zeros(pad, np.int64))
            item_nodes.append(idx_center[wr].T.reshape(-1))
            base += w
        pos_nodes = np.concatenate(pos_nodes)
        item_nodes = np.concatenate(item_nodes)
        dest, nslots, slot_node = _scatter_plan(item_nodes)
        plans.append((dest, nslots, slot_node, pos_nodes))
    nslots_max = max(p[1] for p in plans)
    NBLK = nslots_max // 128
    NBLKW = sum((NJ * w + 127) // 128 for w in chunks)

    w16 = np.zeros((128, 3328), np.float16)
    w32v = np.zeros((128, 64), np.float32)
    col = [0]

    def put16(parts, arr):
        cc = arr.shape[1]
        w16[:parts, col[0]:col[0] + cc] = arr.astype(np.float16)
        col[0] += cc
    col32 = [0]

    def put32(arr):
        arr = np.asarray(arr, np.float32)
        if arr.ndim == 1:
            if arr.shape[0] == 256:
                arr = np.stack([arr[:128], arr[128:]], 1)
            else:
                arr = arr[:, None]
        w32v[:, col32[0]:col32[0] + arr.shape[1]] = arr
        col32[0] += arr.shape[1]

    for li, p in enumerate(params["layers"]):
        W1 = P32(p["conv1"])
        if li == 0:
            put16(64, W1.T)
        else:
            put16(128, W1[:, :128].T)
            put16(32, W1[:, 128:160].T)
        put16(128, P32(p["conv3"]).T)
        put16(128, P32(p["out1"]))
        O2 = P32(p["out2"])
        put16(128, O2[:128])
        put16(128, O2[128:])
    put16(32, P32(params["layers"][0]["rescale"]))
    for vp in params["vns"]:
        put16(128, P32(vp["lin1"]))
        put16(128, P32(vp["lin2"]))
    for li, p in enumerate(params["layers"]):
        put32(P32(p["bn1_g"])); put32(P32(p["bn1_b"]))
        put32(P32(p["obn_g"])); put32(P32(p["obn_b"]))
        put32(P32(p["conv2"]))
    for vp in params["vns"]:
        put32(P32(vp["bn_g"])); put32(P32(vp["bn_b"]))
    put32(P32(params["fin_g"])); put32(P32(params["fin_b"]))

    per_core = []
    for c in range(NC):
        dest, nslots, slot_node, pos_nodes = plans[c]
        w0 = c * WSH
        d = {}
        d["gidx"] = g2row(pos_nodes).reshape(NCALLS, 128).T.astype(np.int32).copy()
        wid = np.zeros(NBLKW * 128, np.int64)
        wid[:NIT] = dest
        pad_ix = np.arange(NIT, NBLKW * 128)
        wid[pad_ix] = nslots_max + (pad_ix % 128)
        d["widx"] = wid.reshape(NBLKW, 128).T.astype(np.int32).copy()
        sn = np.full(NBLK * 128, -1, np.int64)
        sn[:len(slot_node)] = slot_node
        rid = np.where(sn >= 0, g2row(np.maximum(sn, 0)), NREAL)
        d["ridx"] = rid.reshape(NBLK, 128).T.astype(np.int32).copy()
        S = np.zeros((NBLK, 128, 128), np.float16)
        for b in range(NBLK):
            nb = sn[b * 128:(b + 1) * 128]
            eq = (nb[:, None] == nb[None, :]) & (nb[:, None] >= 0)
            wgt = recip[np.maximum(nb, 0)] * (nb >= 0)
            S[b] = (eq * wgt[None, :]).astype(np.float16)
        d["ssrt"] = S
        x0 = np.zeros((64, NPOSP), np.float16)
        base = 0
        posi = 0
        for w in chunks:
            wr = np.arange(w0 + base, w0 + base + w)
            npos = 17 * w
            nposp = ((npos + 127) // 128) * 128
            x0[0:32, posi:posi + npos] = x[wn[wr].T.reshape(-1)].T.astype(np.float16)
            x0[32:64, posi:posi + npos] = ew_full[wr].transpose(1, 0, 2).reshape(-1, 32).T.astype(np.float16)
            base += w
            posi += nposp
        d["xin0"] = x0
        xts = np.zeros((32, NSLICE), np.float16)
        xts[:, :NREAL] = x[c * NREAL:(c + 1) * NREAL].T.astype(np.float16)
        d["xts"] = xts
        Bm = np.zeros((NSLICE, N_GRAPHS), np.float16)
        Bm[np.arange(NREAL), batch[c * NREAL:(c + 1) * NREAL]] = 1.0
        d["bmat"] = Bm
        d["btm"] = Bm.T.copy()
        d["idn"] = np.eye(128, dtype=np.float16)
        d["w16"] = w16
        d["w32"] = w32v
        per_core.append(d)
    return per_core, nslots_max


def kernel(**inputs):
    per_core, nslots_max = make_inmaps(inputs)
    repeat = int(os.environ.get("KREPEAT", "1"))
    nc = _build(nslots_max, repeat=repeat)
    res = run_bass_kernel_spmd(nc, per_core, core_ids=list(range(NC)))
    out = np.zeros((N_NODES, 128), np.float32)
    for c in range(NC):
        out[c * NREAL:(c + 1) * NREAL] = res.results[c]["out_h"][:, :NREAL].T
    return out
